# revision 6
# baseline (speedup 1.0000x reference)
"""MoE layer (gate top-2 + 8 experts + shared expert + LayerNorm) on 8 TRN2 cores.

Strategy: data-parallel over tokens. Each of the 8 cores gets 1024 of the 8192
tokens and computes the full dense MoE for its shard locally (gate, all 8
expert GEMM pairs, shared expert, combine, LayerNorm). No collectives.

Layout notes (per core):
  - x is shipped transposed: xT[d, tok] as [128, D/128, N] so that
    GEMM1 (lhsT=W1 tile [128d, 128h], rhs=xT [128d, 512tok]) produces
    h in [h, tok] layout, which is exactly the lhsT layout GEMM2 needs
    (lhsT=h [128h, 128tok], rhs=W2 [128h, 512do]) -> out in [tok, do].
  - Gate runs in fp32 (routing decisions are precision sensitive);
    expert/shared GEMMs run in bf16 with fp32 PSUM accumulation.
  - Biases b2/shared_b are injected via a K=1 matmul (ones[1,128]^T @ b_row)
    that opens each PSUM accumulation group; b1 rides the gelu activation's
    per-partition bias port.
"""

import numpy as np
import ml_dtypes

BF16 = ml_dtypes.bfloat16

# Problem shapes (hardcoded per contest contract).
B, S, D, E, H, DO = 4, 2048, 1024, 8, 4096, 1024
N_TOK = B * S
N_CORES = 8
P = 128
NB = 512  # matmul moving free-dim / PSUM bank width (fp32)


def build_moe_bass(n_sh=N_TOK // N_CORES, d=D, e_=E, h_=H, do=DO):
    """Build the single-core SPMD Bass program. Returns (nc, out_name)."""
    import concourse.bass as bass
    import concourse.mybir as mybir
    import concourse.tile as tile
    from concourse import bacc

    f32 = mybir.dt.float32
    bf16 = mybir.dt.bfloat16
    AF = mybir.ActivationFunctionType
    ALU = mybir.AluOpType
    X = mybir.AxisListType.X

    DC = d // P        # d chunks
    HC = h_ // P       # h chunks
    TT = n_sh // P     # token tiles of 128
    TH = max(1, n_sh // NB)   # token halves of 512 for GEMM1 rhs
    NBT = min(NB, n_sh)       # token free-dim per GEMM1 matmul
    DH = max(1, do // NB)     # do chunks of 512
    NBD = min(NB, do)         # do free-dim per GEMM2 matmul
    TG = max(1, TT // 4)      # token-tile groups of 4 for GEMM2 psum rotation
    TGS = min(4, TT)          # group size

    nc = bacc.Bacc("TRN2", target_bir_lowering=False)

    xT_t = nc.dram_tensor("xT", [P, DC, n_sh], f32, kind="ExternalInput")
    w1_t = nc.dram_tensor("w1h", [e_, HC, P, DC, P], bf16, kind="ExternalInput")
    w2_t = nc.dram_tensor("w2h", [e_, HC, P, do], bf16, kind="ExternalInput")
    gw_t = nc.dram_tensor("gwr", [P, DC, e_], f32, kind="ExternalInput")
    sw_t = nc.dram_tensor("swh", [P, DC, do], bf16, kind="ExternalInput")
    b1_t = nc.dram_tensor("b1h", [P, e_, HC], f32, kind="ExternalInput")
    b2_t = nc.dram_tensor("b2h", [1, e_, do], bf16, kind="ExternalInput")
    sb_t = nc.dram_tensor("sbh", [1, do], bf16, kind="ExternalInput")
    gam_t = nc.dram_tensor("gam", [do], f32, kind="ExternalInput")
    bet_t = nc.dram_tensor("bet", [do], f32, kind="ExternalInput")
    out_t = nc.dram_tensor("out", [n_sh, do], f32, kind="ExternalOutput")

    with tile.TileContext(nc) as tc:
        with (
            tc.tile_pool(name="resident", bufs=1) as resident,
            tc.tile_pool(name="xtp", bufs=3) as xtp,
            tc.tile_pool(name="w1p", bufs=3) as w1p,
            tc.tile_pool(name="w2p", bufs=4) as w2p,
            tc.tile_pool(name="gatep", bufs=2) as gatep,
            tc.tile_pool(name="lnp", bufs=3) as lnp,
            tc.tile_pool(name="outp", bufs=2) as outp,
            tc.tile_pool(name="psA", bufs=2, space="PSUM") as psA,
            tc.tile_pool(name="psG", bufs=1, space="PSUM") as psG,
            tc.tile_pool(name="psB", bufs=4, space="PSUM") as psB,
        ):
            # ---- resident loads ----
            xbf = resident.tile([P, DC, n_sh], bf16)

            gw = resident.tile([P, DC, e_], f32)
            nc.sync.dma_start(gw[:], gw_t[:])
            sw = resident.tile([P, DC, do], bf16)
            nc.sync.dma_start(sw[:], sw_t[:])
            b1 = resident.tile([P, e_, HC], f32)
            nc.sync.dma_start(b1[:], b1_t[:])
            b2 = resident.tile([1, e_, do], bf16)
            nc.sync.dma_start(b2[:], b2_t[:])
            sb = resident.tile([1, do], bf16)
            nc.sync.dma_start(sb[:], sb_t[:])

            ones = resident.tile([1, P], bf16)
            nc.vector.memset(ones[:], 1.0)
            epst = resident.tile([P, 1], f32)
            nc.vector.memset(epst[:], 1e-5)

            # gamma/beta broadcast across partitions via stride-0 DMA
            gam_bc = resident.tile([P, do], f32)
            g_ap = gam_t[:]
            nc.gpsimd.dma_start(
                out=gam_bc[:],
                in_=bass.AP(tensor=g_ap.tensor, offset=g_ap.offset,
                            ap=[[0, P]] + [list(a) for a in g_ap.ap]),
            )
            bet_bc = resident.tile([P, do], f32)
            b_ap = bet_t[:]
            nc.gpsimd.dma_start(
                out=bet_bc[:],
                in_=bass.AP(tensor=b_ap.tensor, offset=b_ap.offset,
                            ap=[[0, P]] + [list(a) for a in b_ap.ap]),
            )

            # accumulator [tok, do] fp32 and combine weights [tok, e]
            acc = resident.tile([P, TT, do], f32)
            comb = resident.tile([P, TT, e_], f32)

            # ---- gate (fp32) + cast x to bf16, one token-tile at a time ----
            for t in range(TT):
                xt_c = xtp.tile([P, DC, P], f32, tag="xt_c")
                nc.sync.dma_start(xt_c[:], xT_t[:, :, t * P:(t + 1) * P])
                nc.vector.tensor_copy(xbf[:, :, t * P:(t + 1) * P], xt_c[:])
                pg = psG.tile([P, e_], f32, tag="pg")
                for dc in range(DC):
                    nc.tensor.matmul(
                        pg[:], xt_c[:, dc, :], gw[:, dc, :],
                        start=(dc == 0), stop=(dc == DC - 1),
                    )
                lg = gatep.tile([P, e_], f32, tag="lg")
                nc.vector.tensor_copy(lg[:], pg[:])
                m1 = gatep.tile([P, 1], f32, tag="m1")
                nc.vector.reduce_max(m1[:], lg[:], axis=X)
                mask1 = gatep.tile([P, e_], f32, tag="mask1")
                nc.vector.tensor_scalar(mask1[:], lg[:], m1[:], None, ALU.is_ge)
                l2 = gatep.tile([P, e_], f32, tag="l2")
                nc.vector.scalar_tensor_tensor(
                    l2[:], in0=mask1[:], scalar=-1e30, in1=lg[:],
                    op0=ALU.mult, op1=ALU.add,
                )
                m2 = gatep.tile([P, 1], f32, tag="m2")
                nc.vector.reduce_max(m2[:], l2[:], axis=X)
                mask2 = gatep.tile([P, e_], f32, tag="mask2")
                nc.vector.tensor_scalar(mask2[:], l2[:], m2[:], None, ALU.is_ge)
                d21 = gatep.tile([P, 1], f32, tag="d21")
                nc.vector.tensor_sub(d21[:], m2[:], m1[:])
                w2v = gatep.tile([P, 1], f32, tag="w2v")
                nc.scalar.activation(w2v[:], d21[:], AF.Sigmoid)
                w1v = gatep.tile([P, 1], f32, tag="w1v")
                nc.vector.tensor_scalar(w1v[:], w2v[:], -1.0, 1.0, ALU.mult, ALU.add)
                nc.vector.tensor_scalar_mul(mask1[:], mask1[:], w1v[:])
                nc.vector.tensor_scalar_mul(mask2[:], mask2[:], w2v[:])
                nc.vector.tensor_add(comb[:, t, :], mask1[:], mask2[:])

            # ---- shared expert: acc = 0.5*gelu(x @ shared_W + shared_b) ----
            for t in range(TT):
                for dh in range(DH):
                    ps = psB.tile([P, NBD], f32, tag="ps2")
                    nc.tensor.matmul(
                        ps[:], ones[0:1, :], sb[0:1, dh * NBD:(dh + 1) * NBD],
                        start=True, stop=False,
                    )
                    for dc in range(DC):
                        nc.tensor.matmul(
                            ps[:], xbf[:, dc, t * P:(t + 1) * P],
                            sw[:, dc, dh * NBD:(dh + 1) * NBD],
                            start=False, stop=(dc == DC - 1),
                        )
                    a_sl = acc[:, t, dh * NBD:(dh + 1) * NBD]
                    nc.scalar.activation(a_sl, ps[:], AF.Gelu)
                    nc.vector.tensor_scalar_mul(a_sl, a_sl, 0.5)

            # ---- experts ----
            h_sb = resident.tile([P, HC, n_sh], bf16)
            for e in range(e_):
                # GEMM1: h = gelu(x @ W1[e] + b1[e])  in [h, tok] layout
                for hc in range(HC):
                    w1t = w1p.tile([P, DC, P], bf16, tag="w1t")
                    nc.sync.dma_start(w1t[:], w1_t[e, hc])
                    for th in range(TH):
                        ps1 = psA.tile([P, NBT], f32, tag="ps1")
                        for dc in range(DC):
                            nc.tensor.matmul(
                                ps1[:], w1t[:, dc, :],
                                xbf[:, dc, th * NBT:(th + 1) * NBT],
                                start=(dc == 0), stop=(dc == DC - 1),
                            )
                        nc.scalar.activation(
                            h_sb[:, hc, th * NBT:(th + 1) * NBT], ps1[:],
                            AF.Gelu, bias=b1[:, e, hc:hc + 1], scale=1.0,
                        )
                # GEMM2: acc += combine[:, e] * (h @ W2[e] + b2[e])
                for dh in range(DH):
                    for tg in range(TG):
                        pss = [psB.tile([P, NBD], f32, tag="ps2", name=f"ps2_{i}")
                               for i in range(TGS)]
                        for i in range(TGS):
                            nc.tensor.matmul(
                                pss[i][:], ones[0:1, :],
                                b2[0:1, e, dh * NBD:(dh + 1) * NBD],
                                start=True, stop=False,
                            )
                        for hc in range(HC):
                            w2t = w2p.tile([P, NBD], bf16, tag="w2t")
                            nc.sync.dma_start(
                                w2t[:], w2_t[e, hc, :, dh * NBD:(dh + 1) * NBD]
                            )
                            for i in range(TGS):
                                t = tg * TGS + i
                                nc.tensor.matmul(
                                    pss[i][:], h_sb[:, hc, t * P:(t + 1) * P],
                                    w2t[:], start=False, stop=(hc == HC - 1),
                                )
                        for i in range(TGS):
                            t = tg * TGS + i
                            a_sl = acc[:, t, dh * NBD:(dh + 1) * NBD]
                            nc.vector.scalar_tensor_tensor(
                                a_sl, in0=pss[i][:], scalar=comb[:, t, e:e + 1],
                                in1=a_sl, op0=ALU.mult, op1=ALU.add,
                            )

            # ---- LayerNorm over do, then write out ----
            n_sub = do // 512 if do % 512 == 0 and do > 512 else 1
            sub = do // n_sub
            for t in range(TT):
                a_t = acc[:, t, :]
                st = lnp.tile([P, n_sub, 6], f32, tag="st")
                a_view = a_t.rearrange("p (s d) -> p s d", s=n_sub)
                for s in range(n_sub):
                    nc.vector.bn_stats(st[:, s, :], a_view[:, s, :])
                mv = lnp.tile([P, 2], f32, tag="mv")
                nc.vector.bn_aggr(mv[:], st[:])
                rstd = lnp.tile([P, 1], f32, tag="rstd")
                nc.scalar.activation(rstd[:], mv[:, 1:2], AF.Sqrt,
                                     bias=epst[:, 0:1], scale=1.0)
                nc.vector.reciprocal(rstd[:], rstd[:])
                o_t = outp.tile([P, do], f32, tag="o_t")
                nc.vector.tensor_scalar_sub(o_t[:], a_t, mv[:, 0:1])
                nc.vector.scalar_tensor_tensor(
                    o_t[:], in0=o_t[:], scalar=rstd[:], in1=gam_bc[:],
                    op0=ALU.mult, op1=ALU.mult,
                )
                nc.vector.tensor_add(o_t[:], o_t[:], bet_bc[:])
                nc.sync.dma_start(out_t[t * P:(t + 1) * P, :], o_t[:])

    nc.compile()
    return nc


def prep_inputs(x, W1, b1, W2, b2, gate_W, shared_W, shared_b, gamma, beta,
                n_cores=N_CORES):
    """Host-side shard + relayout. Returns list of per-core in_maps."""
    n_tok = int(np.prod(x.shape[:-1]))
    d = x.shape[-1]
    e_, _, h_ = W1.shape
    do = W2.shape[-1]
    n_sh = n_tok // n_cores
    DC, HC = d // P, h_ // P

    flat = np.ascontiguousarray(np.asarray(x, dtype=np.float32).reshape(n_tok, d))
    # weights: shared across cores (runtime copies per core)
    w1h = np.ascontiguousarray(
        np.asarray(W1, dtype=np.float32)
        .reshape(e_, DC, P, HC, P)
        .transpose(0, 3, 2, 1, 4)
    ).astype(BF16)                                     # [e, hc, p, dc, j]
    w2h = np.ascontiguousarray(
        np.asarray(W2, dtype=np.float32).reshape(e_, HC, P, do)
    ).astype(BF16)                                     # [e, hc, p, do]
    gwr = np.ascontiguousarray(
        np.asarray(gate_W, dtype=np.float32).reshape(DC, P, e_).transpose(1, 0, 2)
    )                                                  # [p, dc, e]
    swh = np.ascontiguousarray(
        np.asarray(shared_W, dtype=np.float32).reshape(DC, P, do).transpose(1, 0, 2)
    ).astype(BF16)                                     # [p, dc, do]
    b1h = np.ascontiguousarray(
        np.asarray(b1, dtype=np.float32).reshape(e_, HC, P).transpose(2, 0, 1)
    )                                                  # [p, e, hc]
    b2h = np.asarray(b2, dtype=np.float32).reshape(1, e_, do).astype(BF16)
    sbh = np.asarray(shared_b, dtype=np.float32).reshape(1, do).astype(BF16)
    gam = np.asarray(gamma, dtype=np.float32).reshape(do)
    bet = np.asarray(beta, dtype=np.float32).reshape(do)

    in_maps = []
    for c in range(n_cores):
        shard = flat[c * n_sh:(c + 1) * n_sh]          # [n_sh, d]
        xT = np.ascontiguousarray(
            shard.T.reshape(DC, P, n_sh).transpose(1, 0, 2)
        )                                              # [p, dc, tok]
        in_maps.append({
            "xT": xT, "w1h": w1h, "w2h": w2h, "gwr": gwr, "swh": swh,
            "b1h": b1h, "b2h": b2h, "sbh": sbh, "gam": gam, "bet": bet,
        })
    return in_maps


_NC_CACHE = {}


def kernel(x, W1, b1, W2, b2, gate_W, shared_W, shared_b, gamma, beta):
    from concourse.bass_utils import run_bass_kernel_spmd

    n_tok = int(np.prod(x.shape[:-1]))
    n_sh = n_tok // N_CORES
    key = (n_sh, x.shape[-1])
    if key not in _NC_CACHE:
        _NC_CACHE[key] = build_moe_bass(n_sh=n_sh, d=x.shape[-1],
                                        e_=W1.shape[0], h_=W1.shape[2],
                                        do=W2.shape[-1])
    nc = _NC_CACHE[key]
    in_maps = prep_inputs(x, W1, b1, W2, b2, gate_W, shared_W, shared_b,
                          gamma, beta)
    res = run_bass_kernel_spmd(nc, in_maps, core_ids=list(range(N_CORES)))
    outs = [r["out"] for r in res.results]
    full = np.concatenate(outs, axis=0)               # [n_tok, do]
    return full.reshape(*x.shape[:-1], full.shape[-1]).astype(np.float32)
